# revision 6
# baseline (speedup 1.0000x reference)
"""Trainium2 Bass kernel for nn_Decode (CenterNet-style polygon decode).

8 NeuronCores SPMD, half-image per core: core c convolves image c//2's
half c%2 (conv3x3 64->256 + relu -> conv1x1 256->64 on the PE, weights
stationary, 6 shift-pair/tap matmuls per 512-px tile).  conv2 is computed
transposed (pixel-partition output, bias folded in as a K=1 ones-row
matmul) so the feature map leaves the device channel-last, ready for the
host gather.  Activations cross the (slow, ~30-60 MB/s) axon tunnel in
fp8e4m3 both directions; weights are content-hash cached as device-resident
arrays so warm calls upload only the 4.3MB of activation slabs and download
only the 4.2MB feature map.  The runner is a persistent jit built once
(run_bass_kernel_spmd re-traces and re-uploads donated zero outputs every
call, which dominated the old wall time).  Host: init_polys math (trivial),
numba bilinear sampling of the channel-last feature map, and the fused
(fuse_w@poly_w) refine GEMM with column permutation folded in.
(Device gather primitives are unusable in this container: indirect DMA
silently no-ops under bass2jax/PJRT; dma_gather's Q7 library load fails
codegen outside Bacc - hence the host-side sampling.)
"""
import sys
sys.path.insert(0, '/opt/trn_rl_repo')
import numpy as np
import ml_dtypes

import concourse.bass as bass
import concourse.mybir as mybir
import concourse.tile as tile

F32 = mybir.dt.float32
BF16 = mybir.dt.bfloat16
FP16 = mybir.dt.float16
FP8 = mybir.dt.float8e4
ALU = mybir.AluOpType
ACTF = mybir.ActivationFunctionType
BF = ml_dtypes.bfloat16
F8 = ml_dtypes.float8_e4m3

P = 128
NCORES = 8
B, C, H, W = 4, 64, 128, 128
GRID = 130
SLAB = 66                   # slab rows per core: 64 out rows + 2 halo
XLEN = SLAB * GRID          # 8580
XPAD = 8640
WSCALE = 16.0               # fp8 weight/output scaling
_cache = {}


def _rework_ap(base_ap, extra_off, dims):
    return bass.AP(tensor=base_ap.tensor, offset=base_ap.offset + extra_off, ap=dims)


def build_nc():
    nc = bass.Bass()
    x_in = nc.dram_tensor("x_in", [64, XLEN], FP8, kind="ExternalInput")
    w1 = nc.dram_tensor("w1", [128, 6, 2, 128], FP8, kind="ExternalInput")
    b1 = nc.dram_tensor("b1", [128, 2], F32, kind="ExternalInput")
    w2 = nc.dram_tensor("w2", [128, 2, 64], BF16, kind="ExternalInput")
    b2r = nc.dram_tensor("b2r", [1, 64], BF16, kind="ExternalInput")
    o_f = nc.dram_tensor("o_f", [64, 128, 64], FP8, kind="ExternalOutput")

    with tile.TileContext(nc) as tc:
        with tc.tile_pool(name="persist", bufs=1) as pp:
            w1_sb = pp.tile([128, 6, 2, 128], FP8)
            b1_sb = pp.tile([128, 2], F32)
            w2_sb = pp.tile([128, 2, 64], BF16)
            b2_sb = pp.tile([1, 64], BF16)
            ones = pp.tile([1, 128], BF16)
            x_sb = pp.tile([128, XPAD], FP8)
            fT = pp.tile([128, 4096], FP8)
            scr = pp.tile([128, 4], F32)

            nc.sync.dma_start(w1_sb[:], w1[:])
            nc.sync.dma_start(b1_sb[:], b1[:])
            nc.sync.dma_start(w2_sb[:], w2[:])
            nc.sync.dma_start(b2_sb[:], b2r[:])
            nc.vector.memset(ones[:], 1.0)
            # zero the tail so the +1-shift DMA reads 0 past the slab end
            nc.vector.memset(x_sb[0:64, XLEN - 130:XPAD], 0.0)
            nc.sync.dma_start(x_sb[0:64, 0:XLEN], x_in[:])
            # build the +1-shifted copy in partitions 64:127 on device (SWDGE):
            # dst[64+p, k] = x[p, k+1].  (HWDGE/scalar issue of this copy
            # hard-crashes the exec unit; keep it on gpsimd.)
            xa0 = x_sb[:]
            ps0 = xa0.ap[0][0]
            sh_src = _rework_ap(xa0, 1, [[ps0, 64], [GRID, SLAB], [1, GRID]])
            sh_dst = _rework_ap(xa0, 64 * ps0, [[ps0, 64], [GRID, SLAB], [1, GRID]])
            nc.gpsimd.dma_start(sh_dst, sh_src)

            # conv: 16 tiles of 4 output rows x 128 cols
            PAIR_BASE = [-131, -1, 129]
            with tc.tile_pool(name="conv", bufs=4) as cp, \
                 tc.tile_pool(name="cps", bufs=2, space="PSUM") as cps, \
                 tc.tile_pool(name="cps2", bufs=4, space="PSUM") as cps2:
                xa = x_sb[:]
                pstep = xa.ap[0][0]
                for t in range(16):
                    pbase = (4 * t + 1) * GRID + 1
                    f1t = []
                    for half in range(2):
                        ps = cps.tile([128, 512], F32, space="PSUM", tag="c1")
                        first = True
                        for s, db in enumerate(PAIR_BASE):
                            rhs = _rework_ap(xa, pbase + db,
                                             [[pstep, 128], [GRID, 4], [1, 128]])
                            nc.tensor.matmul(ps[:], w1_sb[:, s, half, :], rhs,
                                             start=first, stop=False,
                                             skip_group_check=not first)
                            first = False
                        rhs3 = _rework_ap(xa, pbase - 129,
                                          [[pstep, 128], [GRID, 4], [1, 128]])
                        nc.tensor.matmul(ps[:], w1_sb[:, 3, half, :], rhs3,
                                         start=False, stop=False,
                                         skip_group_check=True)
                        rhs4 = _rework_ap(xa, pbase,
                                          [[pstep, 128], [GRID, 4], [1, 128]])
                        nc.tensor.matmul(ps[:], w1_sb[:, 4, half, :], rhs4,
                                         start=False, stop=False,
                                         skip_group_check=True)
                        rhs5 = _rework_ap(xa, pbase + 131,
                                          [[pstep, 128], [GRID, 4], [1, 128]])
                        nc.tensor.matmul(ps[:], w1_sb[:, 5, half, :], rhs5,
                                         start=False, stop=True,
                                         skip_group_check=True)
                        f1 = cp.tile([128, 512], BF16, tag=f"f1{half}")
                        nc.scalar.activation(f1[:], ps[:], ACTF.Relu,
                                             bias=b1_sb[:, half:half + 1],
                                             scale=1.0 / WSCALE)
                        f1t.append(f1)
                    # conv2 transposed: out[px, ch]; f1 chunk stationary,
                    # w2 moving; bias via K=1 ones-row matmul
                    for j in range(4):
                        ps2 = cps2.tile([128, 64], F32, space="PSUM", tag="c2")
                        nc.tensor.matmul(ps2[:], ones[:], b2_sb[:],
                                         start=True, stop=False)
                        nc.tensor.matmul(ps2[:], f1t[0][:, 128 * j:128 * (j + 1)],
                                         w2_sb[:, 0, :], start=False, stop=False,
                                         skip_group_check=True)
                        nc.tensor.matmul(ps2[:], f1t[1][:, 128 * j:128 * (j + 1)],
                                         w2_sb[:, 1, :], start=False, stop=True,
                                         skip_group_check=True)
                        R = 4 * t + j
                        nc.scalar.activation(fT[:, R * 64:(R + 1) * 64], ps2[:],
                                             ACTF.Copy, scale=WSCALE)

            # store fT -> o_f[(row, col, ch)]: partition p is the column,
            # free dim is (row, ch); 64B-contiguous bursts on the DRAM side
            dst = _rework_ap(o_f[:], 0, [[64, 128], [128 * 64, 64], [1, 64]])
            nc.scalar.activation(scr[0:128, 0:1], fT[:, 0:1], ACTF.Copy)
            nc.scalar.dma_start(dst, fT[:])
    _split_waits(nc)
    return nc


_SEQ_OK = ('InstUnconditionalBranch', 'InstNoOp', 'InstEventSemaphoreOp')


def _split_waits(nc, limit=1):
    """Walrus wait-slot limits: move multi-waits onto injected NoOps."""
    nid = [0]
    for f in nc.m.functions:
        for bb in f.blocks:
            il = bb.instructions
            out = []
            for ins in il:
                si = ins.sync_info
                nm = ins.__class__.__name__
                if (si is not None and len(si.on_wait) > limit
                        and nm not in _SEQ_OK):
                    waits = list(si.on_wait)
                    for k in range(0, len(waits), 1):
                        no = mybir.InstNoOp(name=f"I-wsplit{nid[0]}", ins=[], outs=[])
                        nid[0] += 1
                        no.engine = ins.engine
                        no.sync_info = mybir.SyncInfo(on_wait=waits[k:k + 1], on_update=[])
                        out.append(no)
                    ins.sync_info = mybir.SyncInfo(on_wait=[], on_update=list(si.on_update))
                out.append(ins)
            il[:] = out


def _get_runner():
    if 'runner' in _cache:
        return _cache['runner']
    import jax
    from jax.experimental.shard_map import shard_map
    from jax.sharding import Mesh, PartitionSpec, NamedSharding
    from concourse import bass2jax
    bass2jax.install_neuronx_cc_hook()

    nc = build_nc()
    partition_name = nc.partition_id_tensor.name if nc.partition_id_tensor else None
    in_names, out_names, out_avals = [], [], []
    for alloc in nc.m.functions[0].allocations:
        if not isinstance(alloc, mybir.MemoryLocationSet):
            continue
        name = alloc.memorylocations[0].name
        if alloc.kind == "ExternalInput":
            if name != partition_name:
                in_names.append(name)
        elif alloc.kind == "ExternalOutput":
            out_names.append(name)
            out_avals.append(jax.core.ShapedArray(
                tuple(alloc.tensor_shape), mybir.dt.np(alloc.dtype)))
    all_names = in_names + out_names
    if partition_name is not None:
        all_names = all_names + [partition_name]

    def _body(*args):
        operands = list(args)
        if partition_name is not None:
            operands.append(bass2jax.partition_id_tensor())
        outs = bass2jax._bass_exec_p.bind(
            *operands,
            out_avals=tuple(out_avals),
            in_names=tuple(all_names),
            out_names=tuple(out_names),
            lowering_input_output_aliases=(),
            sim_require_finite=True,
            sim_require_nnan=True,
            nc=nc,
        )
        return tuple(outs)

    devices = jax.devices()[:NCORES]
    mesh = Mesh(np.asarray(devices), ("core",))
    nargs = len(in_names) + len(out_names)
    fn = jax.jit(shard_map(
        _body, mesh=mesh,
        in_specs=(PartitionSpec("core"),) * nargs,
        out_specs=(PartitionSpec("core"),) * len(out_names)),
        keep_unused=True)
    sharding = NamedSharding(mesh, PartitionSpec("core"))
    runner = {'fn': fn, 'in_names': in_names, 'out_names': out_names,
              'out_avals': out_avals, 'sharding': sharding, 'jax': jax}
    _cache['runner'] = runner
    return runner


def _dev_weights(inputs, runner):
    """Device-resident tiled weight arrays, re-uploaded only when changed."""
    w1 = np.asarray(inputs['conv1_w'], np.float32)
    b1 = np.asarray(inputs['conv1_b'], np.float32)
    w2 = np.asarray(inputs['conv2_w'], np.float32)
    b2 = np.asarray(inputs['conv2_b'], np.float32)
    cw = _cache.get('cw')
    if cw is not None and all(np.array_equal(a, b) for a, b in
                              zip(cw, (w1, b1, w2, b2))):
        return _cache['wdev']
    jax = runner['jax']
    sh = runner['sharding']

    w1r = w1.reshape(256, 64, 3, 3)

    def tapw(dy, dx):
        return w1r[:, :, dy + 1, dx + 1]             # [256, 64]
    w1_dev = np.zeros((128, 6, 2, 128), np.float32)
    pairs = [((-1, -1), (-1, 0)), ((0, -1), (0, 0)), ((1, -1), (1, 0))]
    for s, (ta, tb) in enumerate(pairs):
        for half in range(2):
            w1_dev[0:64, s, half, :] = tapw(*ta)[128 * half:128 * (half + 1)].T
            w1_dev[64:128, s, half, :] = tapw(*tb)[128 * half:128 * (half + 1)].T
    for half in range(2):
        w1_dev[0:64, 3, half, :] = tapw(-1, 1)[128 * half:128 * (half + 1)].T
        w1_dev[64:128, 4, half, :] = tapw(0, 1)[128 * half:128 * (half + 1)].T
        w1_dev[0:64, 5, half, :] = tapw(1, 1)[128 * half:128 * (half + 1)].T
    w1_dev = (w1_dev * WSCALE).astype(F8)
    w2t = w2.reshape(64, 256).T                      # [256, 64]
    w2_dev = np.ascontiguousarray(
        np.stack([w2t[0:128], w2t[128:256]], axis=1)).astype(BF)
    b1_dev = np.stack([b1[0:128], b1[128:256]], 1).astype(np.float32)
    b2_dev = b2.reshape(1, 64).astype(BF)

    def tile8(a):
        g = np.ascontiguousarray(np.broadcast_to(
            a[None], (NCORES,) + a.shape)).reshape((NCORES * a.shape[0],) + a.shape[1:])
        return jax.device_put(g, sh)
    wdev = {'w1': tile8(w1_dev), 'b1': tile8(b1_dev),
            'w2': tile8(w2_dev), 'b2r': tile8(b2_dev)}
    if 'zeros' not in _cache:
        _cache['zeros'] = jax.device_put(
            np.zeros((NCORES * 64, 128, 64), F8), sh)
    wdev['o_f'] = _cache['zeros']
    _cache['cw'] = (w1.copy(), b1.copy(), w2.copy(), b2.copy())
    _cache['wdev'] = wdev
    return wdev


def _refine_w(inputs):
    """Permuted+scaled (fuse_w @ poly_w) [256, 8256]: column q = pt*64+ch
    maps to original column ch*129+pt; scaled by 1/WSCALE (fp8 f export)."""
    pw = np.asarray(inputs['poly_w'], np.float32)
    fw = np.asarray(inputs['fuse_w'], np.float32)
    cr = _cache.get('cr')
    if cr is not None and np.array_equal(cr[0], pw) and np.array_equal(cr[1], fw):
        return _cache['wfq']
    wf = fw @ pw                                      # [256, 8256]
    q = np.arange((P + 1) * 64)
    perm = (q % 64) * (P + 1) + (q // 64)
    wfq = np.ascontiguousarray(wf[:, perm]) * (1.0 / WSCALE)
    _cache['cr'] = (pw.copy(), fw.copy())
    _cache['wfq'] = wfq
    return wfq


_NUMBA_SRC = r'''
import numba
import numpy as np


@numba.njit(fastmath=True, cache=False)
def bilin8(f8, lut, px, py, out):
    """Branchless 4-tap bilinear over one image, fp8 bytes decoded via LUT.
    f8: uint8[128*128*64] channel-last; px/py: f32[m]; out: f32[m, 64]."""
    M = px.shape[0]
    for i in range(M):
        x = px[i] - 0.5
        y = py[i] - 0.5
        x0 = np.floor(x)
        y0 = np.floor(y)
        wx = x - x0
        wy = y - y0
        x0i = int(x0)
        y0i = int(y0)
        vx0 = 0 <= x0i < 128
        vx1 = 0 <= x0i + 1 < 128
        vy0 = 0 <= y0i < 128
        vy1 = 0 <= y0i + 1 < 128
        xc0 = x0i if vx0 else 0
        xc1 = x0i + 1 if vx1 else 0
        yr0 = (y0i << 7) if vy0 else 0
        yr1 = ((y0i + 1) << 7) if vy1 else 0
        w00 = (1.0 - wx) * (1.0 - wy) if (vx0 and vy0) else 0.0
        w01 = wx * (1.0 - wy) if (vx1 and vy0) else 0.0
        w10 = (1.0 - wx) * wy if (vx0 and vy1) else 0.0
        w11 = wx * wy if (vx1 and vy1) else 0.0
        r00 = (yr0 + xc0) << 6
        r01 = (yr0 + xc1) << 6
        r10 = (yr1 + xc0) << 6
        r11 = (yr1 + xc1) << 6
        for ch in range(64):
            out[i, ch] = (w00 * lut[f8[r00 + ch]] + w01 * lut[f8[r01 + ch]]
                          + w10 * lut[f8[r10 + ch]] + w11 * lut[f8[r11 + ch]])
'''


def _get_bilin():
    if 'bilin' not in _cache:
        ns = {}
        exec(compile(_NUMBA_SRC, '<bilin>', 'exec'), ns)
        _cache['bilin'] = ns['bilin8']
        _cache['lut'] = np.arange(256, dtype=np.uint8).view(F8).astype(np.float32)
        _cache['lut'][np.isnan(_cache['lut'])] = 0.0
    return _cache['bilin']


def _bufs():
    if 'bufs' not in _cache:
        _cache['bufs'] = {
            'X': np.zeros((NCORES, 64, SLAB, GRID), F8),
            'f8': np.empty((H, W, 64), np.uint8),
        }
    return _cache['bufs']


def _prep_x(cnn):
    """[8*64, XLEN] fp8 slabs: core 2b+h gets image b's padded rows 64h..64h+65."""
    cnn8 = cnn.astype(F8)
    X = _bufs()['X']
    for b in range(B):
        X[2 * b, :, 1:66, 1:129] = cnn8[b, :, 0:65, :]
        X[2 * b + 1, :, 0:65, 1:129] = cnn8[b, :, 63:128, :]
    return X.reshape(NCORES * 64, XLEN)


def kernel(**inputs):
    import concurrent.futures as cf
    runner = _get_runner()
    bilin = _get_bilin()
    lut = _cache['lut']
    cnn = np.asarray(inputs['cnn_feature'], np.float32)
    wdev = _dev_weights(inputs, runner)
    xg = _prep_x(cnn)
    amap = dict(wdev)
    amap['x_in'] = xg
    args = [amap[n] for n in runner['in_names'] + runner['out_names']]
    out = runner['fn'](*args)                         # async dispatch

    # background per-shard fetch (overlaps the remaining host work)
    ex = _cache.setdefault('ex', cf.ThreadPoolExecutor(1))
    shards = out[0].addressable_shards
    futs = [ex.submit(lambda s=s: np.asarray(s.data)) for s in shards]

    # host-side init polys (trivial math; avoids a device roundtrip)
    wh = np.asarray(inputs['wh_pred'], np.float32)
    ct_ind = np.asarray(inputs['ct_ind'], np.int64)
    ct_img = np.asarray(inputs['ct_img_idx'], np.int64)
    N = ct_ind.shape[0]
    ctx = (ct_ind % W).astype(np.float32)
    cty = (ct_ind // W).astype(np.float32)
    whr = wh[ct_img, :, ct_ind // W, ct_ind % W]      # (N, 2P)
    init = whr.reshape(N, P, 2) * 40.0                # 10 (stride) * 4 (DOWN)
    init[:, :, 0] += 4.0 * ctx[:, None]
    init[:, :, 1] += 4.0 * cty[:, None]

    # sampling points: col 0 = center, cols 1..128 = init_polys (pre-DOWN)
    px = np.empty((N, P + 1), np.float32)
    py = np.empty((N, P + 1), np.float32)
    px[:, 0] = ctx
    py[:, 0] = cty
    px[:, 1:] = init[:, :, 0] * 0.25
    py[:, 1:] = init[:, :, 1] * 0.25

    wfq = _refine_w(inputs)
    fb = np.asarray(inputs['fuse_b'], np.float32)
    coar = np.empty((N, P, 2), np.float32)

    # per-image pipeline: sample + refine image b while b+1 downloads
    f8 = _bufs()['f8']
    for b in range(B):
        idx = np.flatnonzero(ct_img == b)
        f8[0:64] = futs[2 * b].result().view(np.uint8)
        f8[64:128] = futs[2 * b + 1].result().view(np.uint8)
        m = idx.shape[0]
        if m == 0:
            continue
        pxb = np.ascontiguousarray(px[idx]).reshape(-1)
        pyb = np.ascontiguousarray(py[idx]).reshape(-1)
        fp = np.empty((m * (P + 1), 64), np.float32)
        bilin(f8.reshape(-1), lut, pxb, pyb, fp)
        offs = fp.reshape(m, (P + 1) * 64) @ wfq.T + fb
        coar[idx] = offs.reshape(m, P, 2) * 16.0 + init[idx]
    return init, coar


# revision 20
# speedup vs baseline: 2.4543x; 2.4543x over previous
"""Trainium2 Bass kernel for nn_Decode (CenterNet-style polygon decode).

8 NeuronCores SPMD, half-image per core: core c convolves image c//2's
half c%2 (conv3x3 64->256 + relu -> conv1x1 256->64 on the PE, weights
stationary, 6 shift-pair/tap matmuls per 512-px tile).  conv2 is computed
transposed (pixel-partition output, bias folded in as a K=1 ones-row
matmul) so the feature map leaves the device channel-last, ready for the
host gather.  Activations cross the (slow, ~30-60 MB/s) axon tunnel in
fp8e4m3 both directions; weights are content-hash cached as device-resident
arrays so warm calls upload only the 4.3MB of activation slabs and download
only the 4.2MB feature map.  The runner is a persistent jit built once
(run_bass_kernel_spmd re-traces and re-uploads donated zero outputs every
call, which dominated the old wall time).  Host: init_polys math (trivial),
numba bilinear sampling of the channel-last feature map, and the fused
(fuse_w@poly_w) refine GEMM with column permutation folded in.
(Device gather primitives are unusable in this container: indirect DMA
silently no-ops under bass2jax/PJRT; dma_gather's Q7 library load fails
codegen outside Bacc - hence the host-side sampling.)
"""
import sys
sys.path.insert(0, '/opt/trn_rl_repo')
import numpy as np
import ml_dtypes

import concourse.bass as bass
import concourse.mybir as mybir
import concourse.tile as tile

F32 = mybir.dt.float32
BF16 = mybir.dt.bfloat16
FP16 = mybir.dt.float16
FP8 = mybir.dt.float8e4
U8 = mybir.dt.uint8
ALU = mybir.AluOpType
ACTF = mybir.ActivationFunctionType
BF = ml_dtypes.bfloat16
F8 = ml_dtypes.float8_e4m3

P = 128
NCORES = 8
B, C, H, W = 4, 64, 128, 128
GRID = 130
SLAB = 66                   # slab rows per core: 64 out rows + 2 halo
XLEN = SLAB * GRID          # 8580
XPAD = 8640
WSCALE = 16.0               # fp8 weight/output scaling
_cache = {}


def _rework_ap(base_ap, extra_off, dims):
    return bass.AP(tensor=base_ap.tensor, offset=base_ap.offset + extra_off, ap=dims)


def build_nc():
    nc = bass.Bass()
    x_in = nc.dram_tensor("x_in", [64, XLEN], FP8, kind="ExternalInput")
    w1 = nc.dram_tensor("w1", [128, 6, 2, 128], FP8, kind="ExternalInput")
    b1 = nc.dram_tensor("b1", [128, 2], F32, kind="ExternalInput")
    w2 = nc.dram_tensor("w2", [128, 2, 64], BF16, kind="ExternalInput")
    b2r = nc.dram_tensor("b2r", [1, 64], BF16, kind="ExternalInput")
    # 4-bit packed feature map: byte (row, col, chp) = q(ch=2chp) | q(ch=2chp+1)<<4
    o_f = nc.dram_tensor("o_f", [64, 128, 32], U8, kind="ExternalOutput")

    with tile.TileContext(nc) as tc:
        with tc.tile_pool(name="persist", bufs=1) as pp:
            w1_sb = pp.tile([128, 6, 2, 128], FP8)
            b1_sb = pp.tile([128, 2], F32)
            w2_sb = pp.tile([128, 2, 64], BF16)
            b2_sb = pp.tile([1, 64], BF16)
            ones = pp.tile([1, 128], BF16)
            x_sb = pp.tile([128, XPAD], FP8)
            fT = pp.tile([128, 2048], U8)
            scr = pp.tile([128, 4], F32)
            c8 = pp.tile([128, 1], F32)
            nc.vector.memset(c8[:], 8.0)

            nc.sync.dma_start(w1_sb[:], w1[:])
            nc.sync.dma_start(b1_sb[:], b1[:])
            nc.sync.dma_start(w2_sb[:], w2[:])
            nc.sync.dma_start(b2_sb[:], b2r[:])
            nc.vector.memset(ones[:], 1.0)
            # zero the tail so the +1-shift DMA reads 0 past the slab end
            nc.vector.memset(x_sb[0:64, XLEN - 130:XPAD], 0.0)
            nc.sync.dma_start(x_sb[0:64, 0:XLEN], x_in[:])
            # build the +1-shifted copy in partitions 64:127 on device (SWDGE):
            # dst[64+p, k] = x[p, k+1].  (HWDGE/scalar issue of this copy
            # hard-crashes the exec unit; keep it on gpsimd.)
            xa0 = x_sb[:]
            ps0 = xa0.ap[0][0]
            sh_src = _rework_ap(xa0, 1, [[ps0, 64], [GRID, SLAB], [1, GRID]])
            sh_dst = _rework_ap(xa0, 64 * ps0, [[ps0, 64], [GRID, SLAB], [1, GRID]])
            nc.gpsimd.dma_start(sh_dst, sh_src)

            # conv: 16 tiles of 4 output rows x 128 cols
            PAIR_BASE = [-131, -1, 129]
            with tc.tile_pool(name="conv", bufs=4) as cp, \
                 tc.tile_pool(name="cps", bufs=2, space="PSUM") as cps, \
                 tc.tile_pool(name="cps2", bufs=4, space="PSUM") as cps2:
                xa = x_sb[:]
                pstep = xa.ap[0][0]
                for t in range(16):
                    pbase = (4 * t + 1) * GRID + 1
                    f1t = []
                    for half in range(2):
                        ps = cps.tile([128, 512], F32, space="PSUM", tag="c1")
                        first = True
                        for s, db in enumerate(PAIR_BASE):
                            rhs = _rework_ap(xa, pbase + db,
                                             [[pstep, 128], [GRID, 4], [1, 128]])
                            nc.tensor.matmul(ps[:], w1_sb[:, s, half, :], rhs,
                                             start=first, stop=False,
                                             skip_group_check=not first)
                            first = False
                        rhs3 = _rework_ap(xa, pbase - 129,
                                          [[pstep, 128], [GRID, 4], [1, 128]])
                        nc.tensor.matmul(ps[:], w1_sb[:, 3, half, :], rhs3,
                                         start=False, stop=False,
                                         skip_group_check=True)
                        rhs4 = _rework_ap(xa, pbase,
                                          [[pstep, 128], [GRID, 4], [1, 128]])
                        nc.tensor.matmul(ps[:], w1_sb[:, 4, half, :], rhs4,
                                         start=False, stop=False,
                                         skip_group_check=True)
                        rhs5 = _rework_ap(xa, pbase + 131,
                                          [[pstep, 128], [GRID, 4], [1, 128]])
                        nc.tensor.matmul(ps[:], w1_sb[:, 5, half, :], rhs5,
                                         start=False, stop=True,
                                         skip_group_check=True)
                        f1 = cp.tile([128, 512], BF16, tag=f"f1{half}")
                        nc.scalar.activation(f1[:], ps[:], ACTF.Relu,
                                             bias=b1_sb[:, half:half + 1],
                                             scale=1.0 / WSCALE)
                        f1t.append(f1)
                    # conv2 transposed: out[px, ch]; f1 chunk stationary,
                    # w2 moving; bias via K=1 ones-row matmul
                    for j in range(4):
                        ps2 = cps2.tile([128, 64], F32, space="PSUM", tag="c2")
                        nc.tensor.matmul(ps2[:], ones[:], b2_sb[:],
                                         start=True, stop=False)
                        nc.tensor.matmul(ps2[:], f1t[0][:, 128 * j:128 * (j + 1)],
                                         w2_sb[:, 0, :], start=False, stop=False,
                                         skip_group_check=True)
                        nc.tensor.matmul(ps2[:], f1t[1][:, 128 * j:128 * (j + 1)],
                                         w2_sb[:, 1, :], start=False, stop=True,
                                         skip_group_check=True)
                        R = 4 * t + j
                        # 4-bit quantize: q = round(clamp(16 f + 8, 0, 15.49)),
                        # then pack channel pairs q_even | q_odd<<4
                        tq = cp.tile([128, 64], F32, tag="tq")
                        nc.scalar.activation(tq[:], ps2[:], ACTF.Identity,
                                             bias=c8[:], scale=WSCALE)
                        tc2 = cp.tile([128, 64], F32, tag="tc2")
                        nc.vector.tensor_scalar(tc2[:], tq[:], 0.0, 15.49,
                                                ALU.max, ALU.min)
                        uq = cp.tile([128, 64], U8, tag="uq")
                        nc.vector.tensor_copy(uq[:], tc2[:])
                        ua = uq[:]
                        ue = bass.AP(tensor=ua.tensor, offset=ua.offset,
                                     ap=[[ua.ap[0][0], 128], [2, 32]])
                        uo = bass.AP(tensor=ua.tensor, offset=ua.offset + 1,
                                     ap=[[ua.ap[0][0], 128], [2, 32]])
                        po = cp.tile([128, 32], U8, tag="po")
                        nc.vector.tensor_scalar(po[:], uo, 4, None,
                                                ALU.logical_shift_left)
                        nc.vector.tensor_tensor(fT[:, R * 32:(R + 1) * 32],
                                                ue, po[:], ALU.bitwise_or)

            # store fT -> o_f[(row, col, chpair)]: partition p is the column,
            # free dim is (row, chp); 32B-contiguous bursts on the DRAM side
            dst = _rework_ap(o_f[:], 0, [[32, 128], [128 * 32, 64], [1, 32]])
            nc.scalar.activation(scr[0:128, 0:1], fT[:, 0:1], ACTF.Copy)
            nc.scalar.dma_start(dst, fT[:])
    _split_waits(nc)
    return nc


_SEQ_OK = ('InstUnconditionalBranch', 'InstNoOp', 'InstEventSemaphoreOp')


def _split_waits(nc, limit=1):
    """Walrus wait-slot limits: move multi-waits onto injected NoOps."""
    nid = [0]
    for f in nc.m.functions:
        for bb in f.blocks:
            il = bb.instructions
            out = []
            for ins in il:
                si = ins.sync_info
                nm = ins.__class__.__name__
                if (si is not None and len(si.on_wait) > limit
                        and nm not in _SEQ_OK):
                    waits = list(si.on_wait)
                    for k in range(0, len(waits), 1):
                        no = mybir.InstNoOp(name=f"I-wsplit{nid[0]}", ins=[], outs=[])
                        nid[0] += 1
                        no.engine = ins.engine
                        no.sync_info = mybir.SyncInfo(on_wait=waits[k:k + 1], on_update=[])
                        out.append(no)
                    ins.sync_info = mybir.SyncInfo(on_wait=[], on_update=list(si.on_update))
                out.append(ins)
            il[:] = out


def _get_runner():
    if 'runner' in _cache:
        return _cache['runner']
    import jax
    from jax.experimental.shard_map import shard_map
    from jax.sharding import Mesh, PartitionSpec, NamedSharding
    from concourse import bass2jax
    bass2jax.install_neuronx_cc_hook()

    nc = build_nc()
    partition_name = nc.partition_id_tensor.name if nc.partition_id_tensor else None
    in_names, out_names, out_avals = [], [], []
    for alloc in nc.m.functions[0].allocations:
        if not isinstance(alloc, mybir.MemoryLocationSet):
            continue
        name = alloc.memorylocations[0].name
        if alloc.kind == "ExternalInput":
            if name != partition_name:
                in_names.append(name)
        elif alloc.kind == "ExternalOutput":
            out_names.append(name)
            out_avals.append(jax.core.ShapedArray(
                tuple(alloc.tensor_shape), mybir.dt.np(alloc.dtype)))
    all_names = in_names + out_names
    if partition_name is not None:
        all_names = all_names + [partition_name]

    def _body(*args):
        operands = list(args)
        if partition_name is not None:
            operands.append(bass2jax.partition_id_tensor())
        outs = bass2jax._bass_exec_p.bind(
            *operands,
            out_avals=tuple(out_avals),
            in_names=tuple(all_names),
            out_names=tuple(out_names),
            lowering_input_output_aliases=(),
            sim_require_finite=True,
            sim_require_nnan=True,
            nc=nc,
        )
        return tuple(outs)

    devices = jax.devices()[:NCORES]
    mesh = Mesh(np.asarray(devices), ("core",))
    nargs = len(in_names) + len(out_names)
    fn = jax.jit(shard_map(
        _body, mesh=mesh,
        in_specs=(PartitionSpec("core"),) * nargs,
        out_specs=(PartitionSpec("core"),) * len(out_names)),
        keep_unused=True)
    sharding = NamedSharding(mesh, PartitionSpec("core"))
    runner = {'fn': fn, 'in_names': in_names, 'out_names': out_names,
              'out_avals': out_avals, 'sharding': sharding, 'jax': jax}
    _cache['runner'] = runner
    return runner


def _dev_weights(inputs, runner):
    """Device-resident tiled weight arrays, re-uploaded only when changed."""
    w1 = np.asarray(inputs['conv1_w'], np.float32)
    b1 = np.asarray(inputs['conv1_b'], np.float32)
    w2 = np.asarray(inputs['conv2_w'], np.float32)
    b2 = np.asarray(inputs['conv2_b'], np.float32)
    cw = _cache.get('cw')
    if cw is not None and all(np.array_equal(a, b) for a, b in
                              zip(cw, (w1, b1, w2, b2))):
        return _cache['wdev']
    jax = runner['jax']
    sh = runner['sharding']

    w1r = w1.reshape(256, 64, 3, 3)

    def tapw(dy, dx):
        return w1r[:, :, dy + 1, dx + 1]             # [256, 64]
    w1_dev = np.zeros((128, 6, 2, 128), np.float32)
    pairs = [((-1, -1), (-1, 0)), ((0, -1), (0, 0)), ((1, -1), (1, 0))]
    for s, (ta, tb) in enumerate(pairs):
        for half in range(2):
            w1_dev[0:64, s, half, :] = tapw(*ta)[128 * half:128 * (half + 1)].T
            w1_dev[64:128, s, half, :] = tapw(*tb)[128 * half:128 * (half + 1)].T
    for half in range(2):
        w1_dev[0:64, 3, half, :] = tapw(-1, 1)[128 * half:128 * (half + 1)].T
        w1_dev[64:128, 4, half, :] = tapw(0, 1)[128 * half:128 * (half + 1)].T
        w1_dev[0:64, 5, half, :] = tapw(1, 1)[128 * half:128 * (half + 1)].T
    w1_dev = (w1_dev * WSCALE).astype(F8)
    w2t = w2.reshape(64, 256).T                      # [256, 64]
    w2_dev = np.ascontiguousarray(
        np.stack([w2t[0:128], w2t[128:256]], axis=1)).astype(BF)
    b1_dev = np.stack([b1[0:128], b1[128:256]], 1).astype(np.float32)
    b2_dev = b2.reshape(1, 64).astype(BF)

    def tile8(a):
        g = np.ascontiguousarray(np.broadcast_to(
            a[None], (NCORES,) + a.shape)).reshape((NCORES * a.shape[0],) + a.shape[1:])
        return jax.device_put(g, sh)
    wdev = {'w1': tile8(w1_dev), 'b1': tile8(b1_dev),
            'w2': tile8(w2_dev), 'b2r': tile8(b2_dev)}
    if 'zeros' not in _cache:
        _cache['zeros'] = jax.device_put(
            np.zeros((NCORES * 64, 128, 32), np.uint8), sh)
    wdev['o_f'] = _cache['zeros']
    _cache['cw'] = (w1.copy(), b1.copy(), w2.copy(), b2.copy())
    _cache['wdev'] = wdev
    return wdev


def _refine_w(inputs):
    """Permuted+scaled (fuse_w @ poly_w) [256, 8256]: column q = pt*64+ch
    maps to original column ch*129+pt; scaled by 1/WSCALE (fp8 f export)."""
    pw = np.asarray(inputs['poly_w'], np.float32)
    fw = np.asarray(inputs['fuse_w'], np.float32)
    cr = _cache.get('cr')
    if cr is not None and np.array_equal(cr[0], pw) and np.array_equal(cr[1], fw):
        return _cache['wfq']
    wf = fw @ pw                                      # [256, 8256]
    q = np.arange((P + 1) * 64)
    perm = (q % 64) * (P + 1) + (q // 64)
    wfq = np.ascontiguousarray(wf[:, perm])
    _cache['cr'] = (pw.copy(), fw.copy())
    _cache['wfq'] = wfq
    return wfq


_NUMBA_SRC = r'''
import numba
import numpy as np


@numba.njit(fastmath=True, cache=False)
def bilin4(f4, lutlo, luthi, px, py, out):
    """Branchless 4-tap bilinear over one image, 4-bit packed channel pairs.
    f4: uint8[128*128*32] (row, col, chpair); px/py: f32[m]; out: f32[m, 64]."""
    M = px.shape[0]
    for i in range(M):
        x = px[i] - 0.5
        y = py[i] - 0.5
        x0 = np.floor(x)
        y0 = np.floor(y)
        wx = x - x0
        wy = y - y0
        x0i = int(x0)
        y0i = int(y0)
        vx0 = 0 <= x0i < 128
        vx1 = 0 <= x0i + 1 < 128
        vy0 = 0 <= y0i < 128
        vy1 = 0 <= y0i + 1 < 128
        xc0 = x0i if vx0 else 0
        xc1 = x0i + 1 if vx1 else 0
        yr0 = (y0i << 7) if vy0 else 0
        yr1 = ((y0i + 1) << 7) if vy1 else 0
        w00 = (1.0 - wx) * (1.0 - wy) if (vx0 and vy0) else 0.0
        w01 = wx * (1.0 - wy) if (vx1 and vy0) else 0.0
        w10 = (1.0 - wx) * wy if (vx0 and vy1) else 0.0
        w11 = wx * wy if (vx1 and vy1) else 0.0
        r00 = (yr0 + xc0) << 5
        r01 = (yr0 + xc1) << 5
        r10 = (yr1 + xc0) << 5
        r11 = (yr1 + xc1) << 5
        for chp in range(32):
            b00 = f4[r00 + chp]
            b01 = f4[r01 + chp]
            b10 = f4[r10 + chp]
            b11 = f4[r11 + chp]
            out[i, 2 * chp] = (w00 * lutlo[b00] + w01 * lutlo[b01]
                               + w10 * lutlo[b10] + w11 * lutlo[b11])
            out[i, 2 * chp + 1] = (w00 * luthi[b00] + w01 * luthi[b01]
                                   + w10 * luthi[b10] + w11 * luthi[b11])
'''


def _get_bilin():
    if 'bilin' not in _cache:
        ns = {}
        exec(compile(_NUMBA_SRC, '<bilin>', 'exec'), ns)
        _cache['bilin'] = ns['bilin4']
        bb = np.arange(256, dtype=np.uint8)
        _cache['lutlo'] = (((bb & 15).astype(np.float32)) - 8.0) / WSCALE
        _cache['luthi'] = (((bb >> 4).astype(np.float32)) - 8.0) / WSCALE
    return _cache['bilin']


def _bufs():
    if 'bufs' not in _cache:
        _cache['bufs'] = {
            'X': np.zeros((NCORES, 64, SLAB, GRID), F8),
            'f4': np.empty((H, W, 32), np.uint8),
        }
    return _cache['bufs']


def _prep_x(cnn):
    """[8*64, XLEN] fp8 slabs: core 2b+h gets image b's padded rows 64h..64h+65."""
    cnn8 = cnn.astype(F8)
    X = _bufs()['X']
    for b in range(B):
        X[2 * b, :, 1:66, 1:129] = cnn8[b, :, 0:65, :]
        X[2 * b + 1, :, 0:65, 1:129] = cnn8[b, :, 63:128, :]
    return X.reshape(NCORES * 64, XLEN)


def kernel(**inputs):
    import concurrent.futures as cf
    runner = _get_runner()
    bilin = _get_bilin()
    lutlo, luthi = _cache['lutlo'], _cache['luthi']
    cnn = np.asarray(inputs['cnn_feature'], np.float32)
    wdev = _dev_weights(inputs, runner)
    xg = _prep_x(cnn)
    amap = dict(wdev)
    amap['x_in'] = xg
    args = [amap[n] for n in runner['in_names'] + runner['out_names']]
    out = runner['fn'](*args)                         # async dispatch

    # host-side init polys (trivial math; avoids a device roundtrip)
    wh = np.asarray(inputs['wh_pred'], np.float32)
    ct_ind = np.asarray(inputs['ct_ind'], np.int64)
    ct_img = np.asarray(inputs['ct_img_idx'], np.int64)
    N = ct_ind.shape[0]
    ctx = (ct_ind % W).astype(np.float32)
    cty = (ct_ind // W).astype(np.float32)
    whr = wh[ct_img, :, ct_ind // W, ct_ind % W]      # (N, 2P)
    init = whr.reshape(N, P, 2) * 40.0                # 10 (stride) * 4 (DOWN)
    init[:, :, 0] += 4.0 * ctx[:, None]
    init[:, :, 1] += 4.0 * cty[:, None]

    # sampling points: col 0 = center, cols 1..128 = init_polys (pre-DOWN)
    px = np.empty((N, P + 1), np.float32)
    py = np.empty((N, P + 1), np.float32)
    px[:, 0] = ctx
    py[:, 0] = cty
    px[:, 1:] = init[:, :, 0] * 0.25
    py[:, 1:] = init[:, :, 1] * 0.25

    wfq = _refine_w(inputs)
    fb = np.asarray(inputs['fuse_b'], np.float32)
    coar = np.empty((N, P, 2), np.float32)

    # single bulk fetch (each fetch call costs ~75ms fixed), then
    # per-image sample + refine; image b = cores 2b,2b+1 contiguously
    o4 = np.asarray(out[0]).reshape(B, H, W, 32)
    for b in range(B):
        idx = np.flatnonzero(ct_img == b)
        m = idx.shape[0]
        if m == 0:
            continue
        pxb = np.ascontiguousarray(px[idx]).reshape(-1)
        pyb = np.ascontiguousarray(py[idx]).reshape(-1)
        fp = np.empty((m * (P + 1), 64), np.float32)
        bilin(o4[b].reshape(-1), lutlo, luthi, pxb, pyb, fp)
        offs = fp.reshape(m, (P + 1) * 64) @ wfq.T + fb
        coar[idx] = offs.reshape(m, P, 2) * 16.0 + init[idx]
    return init, coar


# revision 30
# speedup vs baseline: 3.1439x; 1.2810x over previous
"""Trainium2 Bass kernel for nn_Decode (CenterNet-style polygon decode).

8 NeuronCores SPMD, half-image per core: core c convolves image c//2's
half c%2 (conv3x3 64->256 + relu -> conv1x1 256->64 on the PE, weights
stationary, 6 shift-pair/tap matmuls per 512-px tile).  conv2 is computed
transposed (pixel-partition output, bias folded in as a K=1 ones-row
matmul) so the feature map leaves the device channel-last, ready for the
host gather.  Activations cross the (slow, ~30-60 MB/s) axon tunnel in
fp8e4m3 both directions; weights are content-hash cached as device-resident
arrays so warm calls upload only the 4.3MB of activation slabs and download
only the 4.2MB feature map.  The runner is a persistent jit built once
(run_bass_kernel_spmd re-traces and re-uploads donated zero outputs every
call, which dominated the old wall time).  Host: init_polys math (trivial),
numba bilinear sampling of the channel-last feature map, and the fused
(fuse_w@poly_w) refine GEMM with column permutation folded in.
(Device gather primitives are unusable in this container: indirect DMA
silently no-ops under bass2jax/PJRT; dma_gather's Q7 library load fails
codegen outside Bacc - hence the host-side sampling.)
"""
import sys
sys.path.insert(0, '/opt/trn_rl_repo')
import numpy as np
import ml_dtypes

import concourse.bass as bass
import concourse.mybir as mybir
import concourse.tile as tile

F32 = mybir.dt.float32
BF16 = mybir.dt.bfloat16
FP16 = mybir.dt.float16
FP8 = mybir.dt.float8e4
U8 = mybir.dt.uint8
ALU = mybir.AluOpType
ACTF = mybir.ActivationFunctionType
BF = ml_dtypes.bfloat16
F8 = ml_dtypes.float8_e4m3

P = 128
NCORES = 8
B, C, H, W = 4, 64, 128, 128
GRID = 130
SLAB = 66                   # slab rows per core: 64 out rows + 2 halo
XLEN = SLAB * GRID          # 8580
XPAD = 8640
WSCALE = 16.0               # fp8 weight scaling
XS = 2.2                    # 4-bit x quantization scale: code = round(x*XS)+8
_cache = {}


def _rework_ap(base_ap, extra_off, dims):
    return bass.AP(tensor=base_ap.tensor, offset=base_ap.offset + extra_off, ap=dims)


def build_nc():
    nc = bass.Bass()
    x_in = nc.dram_tensor("x_in", [64, XLEN // 2], U8, kind="ExternalInput")
    w1 = nc.dram_tensor("w1", [128, 6, 2, 128], FP8, kind="ExternalInput")
    b1 = nc.dram_tensor("b1", [128, 2], F32, kind="ExternalInput")
    w2 = nc.dram_tensor("w2", [128, 2, 64], BF16, kind="ExternalInput")
    b2r = nc.dram_tensor("b2r", [1, 64], BF16, kind="ExternalInput")
    # 4-bit packed feature map: byte (row, col, chp) = q(ch=2chp) | q(ch=2chp+1)<<4
    o_f = nc.dram_tensor("o_f", [64, 128, 32], U8, kind="ExternalOutput")

    with tile.TileContext(nc) as tc:
        with tc.tile_pool(name="persist", bufs=1) as pp:
            w1_sb = pp.tile([128, 6, 2, 128], FP8)
            b1_sb = pp.tile([128, 2], F32)
            w2_sb = pp.tile([128, 2, 64], BF16)
            b2_sb = pp.tile([1, 64], BF16)
            ones = pp.tile([1, 128], BF16)
            x_sb = pp.tile([128, XPAD], FP8)
            fT = pp.tile([128, 2048], U8)
            scr = pp.tile([128, 4], F32)
            c8 = pp.tile([128, 1], F32)
            nc.vector.memset(c8[:], 8.0)

            nc.sync.dma_start(w1_sb[:], w1[:])
            nc.sync.dma_start(b1_sb[:], b1[:])
            nc.sync.dma_start(w2_sb[:], w2[:])
            nc.sync.dma_start(b2_sb[:], b2r[:])
            nc.vector.memset(ones[:], 1.0)
            # zero the tail so the +1-shift DMA reads 0 past the slab end
            nc.vector.memset(x_sb[0:64, XLEN - 130:XPAD], 0.0)
            # 4-bit unpack: byte k = q(2k) | q(2k+1)<<4, value = (q-8)  (true
            # x times XS; the 1/XS folds into the conv1 epilogue scale)
            xq4 = pp.tile([64, XLEN // 2], U8)
            tl = pp.tile([64, XLEN // 2], U8)
            th = pp.tile([64, XLEN // 2], U8)
            nc.sync.dma_start(xq4[:], x_in[:])
            nc.vector.tensor_scalar(tl[:], xq4[:], 15, None, ALU.bitwise_and)
            nc.vector.tensor_scalar(th[:], xq4[:], 4, None, ALU.logical_shift_right)
            xa0 = x_sb[:]
            ps0 = xa0.ap[0][0]
            xe = bass.AP(tensor=xa0.tensor, offset=xa0.offset,
                         ap=[[ps0, 64], [2, XLEN // 2]])
            xo = bass.AP(tensor=xa0.tensor, offset=xa0.offset + 1,
                         ap=[[ps0, 64], [2, XLEN // 2]])
            nc.vector.tensor_scalar(xe, tl[:], 8, None, ALU.subtract)
            nc.vector.tensor_scalar(xo, th[:], 8, None, ALU.subtract)
            # build the +1-shifted copy in partitions 64:127 on device (SWDGE):
            # dst[64+p, k] = x[p, k+1].  (HWDGE/scalar issue of this copy
            # hard-crashes the exec unit; keep it on gpsimd.)
            sh_src = _rework_ap(xa0, 1, [[ps0, 64], [GRID, SLAB], [1, GRID]])
            sh_dst = _rework_ap(xa0, 64 * ps0, [[ps0, 64], [GRID, SLAB], [1, GRID]])
            nc.gpsimd.dma_start(sh_dst, sh_src)

            # conv: 16 tiles of 4 output rows x 128 cols
            PAIR_BASE = [-131, -1, 129]
            with tc.tile_pool(name="conv", bufs=4) as cp, \
                 tc.tile_pool(name="cps", bufs=2, space="PSUM") as cps, \
                 tc.tile_pool(name="cps2", bufs=4, space="PSUM") as cps2:
                xa = x_sb[:]
                pstep = xa.ap[0][0]
                for t in range(16):
                    pbase = (4 * t + 1) * GRID + 1
                    f1t = []
                    for half in range(2):
                        ps = cps.tile([128, 512], F32, space="PSUM", tag="c1")
                        first = True
                        for s, db in enumerate(PAIR_BASE):
                            rhs = _rework_ap(xa, pbase + db,
                                             [[pstep, 128], [GRID, 4], [1, 128]])
                            nc.tensor.matmul(ps[:], w1_sb[:, s, half, :], rhs,
                                             start=first, stop=False,
                                             skip_group_check=not first)
                            first = False
                        rhs3 = _rework_ap(xa, pbase - 129,
                                          [[pstep, 128], [GRID, 4], [1, 128]])
                        nc.tensor.matmul(ps[:], w1_sb[:, 3, half, :], rhs3,
                                         start=False, stop=False,
                                         skip_group_check=True)
                        rhs4 = _rework_ap(xa, pbase,
                                          [[pstep, 128], [GRID, 4], [1, 128]])
                        nc.tensor.matmul(ps[:], w1_sb[:, 4, half, :], rhs4,
                                         start=False, stop=False,
                                         skip_group_check=True)
                        rhs5 = _rework_ap(xa, pbase + 131,
                                          [[pstep, 128], [GRID, 4], [1, 128]])
                        nc.tensor.matmul(ps[:], w1_sb[:, 5, half, :], rhs5,
                                         start=False, stop=True,
                                         skip_group_check=True)
                        f1 = cp.tile([128, 512], BF16, tag=f"f1{half}")
                        nc.scalar.activation(f1[:], ps[:], ACTF.Relu,
                                             bias=b1_sb[:, half:half + 1],
                                             scale=1.0 / (WSCALE * XS))
                        f1t.append(f1)
                    # conv2 transposed: out[px, ch]; f1 chunk stationary,
                    # w2 moving; bias via K=1 ones-row matmul
                    for j in range(4):
                        ps2 = cps2.tile([128, 64], F32, space="PSUM", tag="c2")
                        nc.tensor.matmul(ps2[:], ones[:], b2_sb[:],
                                         start=True, stop=False)
                        nc.tensor.matmul(ps2[:], f1t[0][:, 128 * j:128 * (j + 1)],
                                         w2_sb[:, 0, :], start=False, stop=False,
                                         skip_group_check=True)
                        nc.tensor.matmul(ps2[:], f1t[1][:, 128 * j:128 * (j + 1)],
                                         w2_sb[:, 1, :], start=False, stop=True,
                                         skip_group_check=True)
                        R = 4 * t + j
                        # 4-bit quantize: q = round(clamp(16 f + 8, 0, 15.49)),
                        # then pack channel pairs q_even | q_odd<<4
                        tq = cp.tile([128, 64], F32, tag="tq")
                        nc.scalar.activation(tq[:], ps2[:], ACTF.Identity,
                                             bias=c8[:], scale=WSCALE)
                        tc2 = cp.tile([128, 64], F32, tag="tc2")
                        nc.vector.tensor_scalar(tc2[:], tq[:], 0.0, 15.49,
                                                ALU.max, ALU.min)
                        uq = cp.tile([128, 64], U8, tag="uq")
                        nc.vector.tensor_copy(uq[:], tc2[:])
                        ua = uq[:]
                        ue = bass.AP(tensor=ua.tensor, offset=ua.offset,
                                     ap=[[ua.ap[0][0], 128], [2, 32]])
                        uo = bass.AP(tensor=ua.tensor, offset=ua.offset + 1,
                                     ap=[[ua.ap[0][0], 128], [2, 32]])
                        po = cp.tile([128, 32], U8, tag="po")
                        nc.vector.tensor_scalar(po[:], uo, 4, None,
                                                ALU.logical_shift_left)
                        nc.vector.tensor_tensor(fT[:, R * 32:(R + 1) * 32],
                                                ue, po[:], ALU.bitwise_or)

            # store fT -> o_f[(row, col, chpair)]: partition p is the column,
            # free dim is (row, chp); 32B-contiguous bursts on the DRAM side
            dst = _rework_ap(o_f[:], 0, [[32, 128], [128 * 32, 64], [1, 32]])
            nc.scalar.activation(scr[0:128, 0:1], fT[:, 0:1], ACTF.Copy)
            nc.scalar.dma_start(dst, fT[:])
    _split_waits(nc)
    return nc


_SEQ_OK = ('InstUnconditionalBranch', 'InstNoOp', 'InstEventSemaphoreOp')


def _split_waits(nc, limit=1):
    """Walrus wait-slot limits: move multi-waits onto injected NoOps."""
    nid = [0]
    for f in nc.m.functions:
        for bb in f.blocks:
            il = bb.instructions
            out = []
            for ins in il:
                si = ins.sync_info
                nm = ins.__class__.__name__
                if (si is not None and len(si.on_wait) > limit
                        and nm not in _SEQ_OK):
                    waits = list(si.on_wait)
                    for k in range(0, len(waits), 1):
                        no = mybir.InstNoOp(name=f"I-wsplit{nid[0]}", ins=[], outs=[])
                        nid[0] += 1
                        no.engine = ins.engine
                        no.sync_info = mybir.SyncInfo(on_wait=waits[k:k + 1], on_update=[])
                        out.append(no)
                    ins.sync_info = mybir.SyncInfo(on_wait=[], on_update=list(si.on_update))
                out.append(ins)
            il[:] = out


def _get_runner():
    if 'runner' in _cache:
        return _cache['runner']
    import jax
    from jax.experimental.shard_map import shard_map
    from jax.sharding import Mesh, PartitionSpec, NamedSharding
    from concourse import bass2jax
    bass2jax.install_neuronx_cc_hook()

    nc = build_nc()
    partition_name = nc.partition_id_tensor.name if nc.partition_id_tensor else None
    in_names, out_names, out_avals = [], [], []
    for alloc in nc.m.functions[0].allocations:
        if not isinstance(alloc, mybir.MemoryLocationSet):
            continue
        name = alloc.memorylocations[0].name
        if alloc.kind == "ExternalInput":
            if name != partition_name:
                in_names.append(name)
        elif alloc.kind == "ExternalOutput":
            out_names.append(name)
            out_avals.append(jax.core.ShapedArray(
                tuple(alloc.tensor_shape), mybir.dt.np(alloc.dtype)))
    all_names = in_names + out_names
    if partition_name is not None:
        all_names = all_names + [partition_name]

    def _body(*args):
        operands = list(args)
        if partition_name is not None:
            operands.append(bass2jax.partition_id_tensor())
        outs = bass2jax._bass_exec_p.bind(
            *operands,
            out_avals=tuple(out_avals),
            in_names=tuple(all_names),
            out_names=tuple(out_names),
            lowering_input_output_aliases=(),
            sim_require_finite=True,
            sim_require_nnan=True,
            nc=nc,
        )
        return tuple(outs)

    devices = jax.devices()[:NCORES]
    mesh = Mesh(np.asarray(devices), ("core",))
    nargs = len(in_names) + len(out_names)
    fn = jax.jit(shard_map(
        _body, mesh=mesh,
        in_specs=(PartitionSpec("core"),) * nargs,
        out_specs=(PartitionSpec("core"),) * len(out_names)),
        keep_unused=True)
    sharding = NamedSharding(mesh, PartitionSpec("core"))
    runner = {'fn': fn, 'in_names': in_names, 'out_names': out_names,
              'out_avals': out_avals, 'sharding': sharding, 'jax': jax}
    _cache['runner'] = runner
    return runner


def _dev_weights(inputs, runner):
    """Device-resident tiled weight arrays, re-uploaded only when changed."""
    w1 = np.asarray(inputs['conv1_w'], np.float32)
    b1 = np.asarray(inputs['conv1_b'], np.float32)
    w2 = np.asarray(inputs['conv2_w'], np.float32)
    b2 = np.asarray(inputs['conv2_b'], np.float32)
    cw = _cache.get('cw')
    if cw is not None and all(np.array_equal(a, b) for a, b in
                              zip(cw, (w1, b1, w2, b2))):
        return _cache['wdev']
    jax = runner['jax']
    sh = runner['sharding']

    w1r = w1.reshape(256, 64, 3, 3)

    def tapw(dy, dx):
        return w1r[:, :, dy + 1, dx + 1]             # [256, 64]
    w1_dev = np.zeros((128, 6, 2, 128), np.float32)
    pairs = [((-1, -1), (-1, 0)), ((0, -1), (0, 0)), ((1, -1), (1, 0))]
    for s, (ta, tb) in enumerate(pairs):
        for half in range(2):
            w1_dev[0:64, s, half, :] = tapw(*ta)[128 * half:128 * (half + 1)].T
            w1_dev[64:128, s, half, :] = tapw(*tb)[128 * half:128 * (half + 1)].T
    for half in range(2):
        w1_dev[0:64, 3, half, :] = tapw(-1, 1)[128 * half:128 * (half + 1)].T
        w1_dev[64:128, 4, half, :] = tapw(0, 1)[128 * half:128 * (half + 1)].T
        w1_dev[0:64, 5, half, :] = tapw(1, 1)[128 * half:128 * (half + 1)].T
    w1_dev = (w1_dev * WSCALE).astype(F8)
    w2t = w2.reshape(64, 256).T                      # [256, 64]
    w2_dev = np.ascontiguousarray(
        np.stack([w2t[0:128], w2t[128:256]], axis=1)).astype(BF)
    b1_dev = np.stack([b1[0:128], b1[128:256]], 1).astype(np.float32)
    b2_dev = b2.reshape(1, 64).astype(BF)

    def tile8(a):
        g = np.ascontiguousarray(np.broadcast_to(
            a[None], (NCORES,) + a.shape)).reshape((NCORES * a.shape[0],) + a.shape[1:])
        return jax.device_put(g, sh)
    wdev = {'w1': tile8(w1_dev), 'b1': tile8(b1_dev),
            'w2': tile8(w2_dev), 'b2r': tile8(b2_dev)}
    if 'zeros' not in _cache:
        _cache['zeros'] = jax.device_put(
            np.zeros((NCORES * 64, 128, 32), np.uint8), sh)
    wdev['o_f'] = _cache['zeros']
    _cache['cw'] = (w1.copy(), b1.copy(), w2.copy(), b2.copy())
    _cache['wdev'] = wdev
    return wdev


def _refine_w(inputs):
    """Permuted+scaled (fuse_w @ poly_w) [256, 8256]: column q = pt*64+ch
    maps to original column ch*129+pt; scaled by 1/WSCALE (fp8 f export)."""
    pw = np.asarray(inputs['poly_w'], np.float32)
    fw = np.asarray(inputs['fuse_w'], np.float32)
    cr = _cache.get('cr')
    if cr is not None and np.array_equal(cr[0], pw) and np.array_equal(cr[1], fw):
        return _cache['wfq']
    wf = fw @ pw                                      # [256, 8256]
    q = np.arange((P + 1) * 64)
    perm = (q % 64) * (P + 1) + (q // 64)
    wfq = np.ascontiguousarray(wf[:, perm])
    _cache['cr'] = (pw.copy(), fw.copy())
    _cache['wfq'] = wfq
    return wfq


_NUMBA_SRC = r'''
import numba
import numpy as np


@numba.njit(fastmath=True, cache=False)
def bilin4(f4, lutlo, luthi, px, py, out):
    """Branchless 4-tap bilinear over one image, 4-bit packed channel pairs.
    f4: uint8[128*128*32] (row, col, chpair); px/py: f32[m]; out: f32[m, 64]."""
    M = px.shape[0]
    for i in range(M):
        x = px[i] - 0.5
        y = py[i] - 0.5
        x0 = np.floor(x)
        y0 = np.floor(y)
        wx = x - x0
        wy = y - y0
        x0i = int(x0)
        y0i = int(y0)
        vx0 = 0 <= x0i < 128
        vx1 = 0 <= x0i + 1 < 128
        vy0 = 0 <= y0i < 128
        vy1 = 0 <= y0i + 1 < 128
        xc0 = x0i if vx0 else 0
        xc1 = x0i + 1 if vx1 else 0
        yr0 = (y0i << 7) if vy0 else 0
        yr1 = ((y0i + 1) << 7) if vy1 else 0
        w00 = (1.0 - wx) * (1.0 - wy) if (vx0 and vy0) else 0.0
        w01 = wx * (1.0 - wy) if (vx1 and vy0) else 0.0
        w10 = (1.0 - wx) * wy if (vx0 and vy1) else 0.0
        w11 = wx * wy if (vx1 and vy1) else 0.0
        r00 = (yr0 + xc0) << 5
        r01 = (yr0 + xc1) << 5
        r10 = (yr1 + xc0) << 5
        r11 = (yr1 + xc1) << 5
        for chp in range(32):
            b00 = f4[r00 + chp]
            b01 = f4[r01 + chp]
            b10 = f4[r10 + chp]
            b11 = f4[r11 + chp]
            out[i, 2 * chp] = (w00 * lutlo[b00] + w01 * lutlo[b01]
                               + w10 * lutlo[b10] + w11 * lutlo[b11])
            out[i, 2 * chp + 1] = (w00 * luthi[b00] + w01 * luthi[b01]
                                   + w10 * luthi[b10] + w11 * luthi[b11])


@numba.njit(fastmath=True, cache=False)
def packx(cnn, X4, xs):
    """4-bit pack the padded conv-input slabs.
    cnn: f32[4,64,128,128]; X4: u8[8,64,66,65] prefilled 0x88 (code 8 = 0.0).
    Slab 2b+h covers image b's padded rows 64h..64h+65; byte j of a row
    packs padded cols (2j, 2j+1); interior cols are 1..128."""
    for b in range(4):
        for h in range(2):
            r0 = 63 * h          # first image row of the slab interior
            sr0 = 1 - h          # slab row where the interior starts
            for ch in range(64):
                for r in range(65):
                    src = cnn[b, ch, r0 + r]
                    dst = X4[2 * b + h, ch, sr0 + r]
                    # padded col 0 is pad (code 8); cols 1..128 are pixels
                    prev = 8
                    for j in range(64):
                        v = src[2 * j] * xs + 8.5
                        q = int(v)
                        if v < 0.0:
                            q = 0
                        elif q > 15:
                            q = 15
                        dst[j] = np.uint8(prev | (q << 4))
                        v = src[2 * j + 1] * xs + 8.5
                        q = int(v)
                        if v < 0.0:
                            q = 0
                        elif q > 15:
                            q = 15
                        prev = q
                    dst[64] = np.uint8(prev | (8 << 4))
'''


def _get_bilin():
    if 'bilin' not in _cache:
        ns = {}
        exec(compile(_NUMBA_SRC, '<bilin>', 'exec'), ns)
        _cache['bilin'] = ns['bilin4']
        _cache['packx'] = ns['packx']
        bb = np.arange(256, dtype=np.uint8)
        _cache['lutlo'] = (((bb & 15).astype(np.float32)) - 8.0) / WSCALE
        _cache['luthi'] = (((bb >> 4).astype(np.float32)) - 8.0) / WSCALE
    return _cache['bilin']


def _bufs():
    if 'bufs' not in _cache:
        _cache['bufs'] = {
            'X4': np.full((NCORES, 64, SLAB, XLEN // (2 * SLAB)), 0x88, np.uint8),
            'fp': np.empty((1024 * (P + 1), 64), np.float32),
            'px': np.empty((1024, P + 1), np.float32),
            'py': np.empty((1024, P + 1), np.float32),
        }
    return _cache['bufs']


def _prep_x(cnn):
    """4-bit packed slabs [8*64, XLEN//2]: core 2b+h gets padded rows 64h..64h+65."""
    X4 = _bufs()['X4']
    _cache['packx'](cnn, X4, np.float32(XS))
    return X4.reshape(NCORES * 64, XLEN // 2)


def kernel(**inputs):
    runner = _get_runner()
    bilin = _get_bilin()
    lutlo, luthi = _cache['lutlo'], _cache['luthi']
    cnn = np.asarray(inputs['cnn_feature'], np.float32)
    wdev = _dev_weights(inputs, runner)
    xg = _prep_x(cnn)
    amap = dict(wdev)
    amap['x_in'] = xg
    args = [amap[n] for n in runner['in_names'] + runner['out_names']]
    out = runner['fn'](*args)                         # async dispatch

    # host-side init polys (trivial math; avoids a device roundtrip),
    # computed in image-sorted order so sampling + GEMM run on
    # contiguous blocks; results are scattered back at the end
    wh = np.asarray(inputs['wh_pred'], np.float32)
    ct_ind = np.asarray(inputs['ct_ind'], np.int64)
    ct_img = np.asarray(inputs['ct_img_idx'], np.int64)
    N = ct_ind.shape[0]
    ord_ = np.argsort(ct_img, kind='stable')
    cts = ct_ind[ord_]
    seg = np.searchsorted(ct_img[ord_], np.arange(B + 1))
    ctx = (cts % W).astype(np.float32)
    cty = (cts // W).astype(np.float32)
    whr = wh[ct_img[ord_], :, cts // W, cts % W]      # (N, 2P)
    init_s = whr.reshape(N, P, 2) * 40.0              # 10 (stride) * 4 (DOWN)
    init_s[:, :, 0] += 4.0 * ctx[:, None]
    init_s[:, :, 1] += 4.0 * cty[:, None]

    # sampling points: col 0 = center, cols 1..128 = init_polys (pre-DOWN)
    bufs = _bufs()
    if N == 1024:
        px, py, fp = bufs['px'], bufs['py'], bufs['fp']
    else:
        px = np.empty((N, P + 1), np.float32)
        py = np.empty((N, P + 1), np.float32)
        fp = np.empty((N * (P + 1), 64), np.float32)
    px[:, 0] = ctx
    py[:, 0] = cty
    px[:, 1:] = init_s[:, :, 0] * 0.25
    py[:, 1:] = init_s[:, :, 1] * 0.25

    wfq = _refine_w(inputs)
    fb = np.asarray(inputs['fuse_b'], np.float32)

    # single bulk fetch (each fetch call costs ~75ms fixed), per-image
    # sampling into one buffer, then a single refine GEMM
    o4 = np.asarray(out[0]).reshape(B, H, W, 32)
    for b in range(B):
        lo, hi = seg[b], seg[b + 1]
        if hi == lo:
            continue
        bilin(o4[b].reshape(-1), lutlo, luthi,
              px[lo:hi].reshape(-1), py[lo:hi].reshape(-1),
              fp[lo * (P + 1):hi * (P + 1)])
    offs = fp[:N * (P + 1)].reshape(N, (P + 1) * 64) @ wfq.T + fb
    coar_s = offs.reshape(N, P, 2) * 16.0 + init_s
    init = np.empty_like(init_s)
    coar = np.empty_like(coar_s)
    init[ord_] = init_s
    coar[ord_] = coar_s
    return init, coar
